# revision 1
# baseline (speedup 1.0000x reference)
"""Trainium2 Bass kernel for nn_CONCATNet_7447473291796 (gnn_message_passing).

Strategy (pure data parallelism, 16 batches per core across 8 cores):
  Only 64 of the 4096 wafer rows per batch feed the PM embeddings, so the
  kernel gathers them straight from the HBM-resident batch shard with
  mainline indirect DMA (InstDMACopy + DynamicAccessPattern on the SWDGE
  software queue): 8 calls x 128 rows, one row per partition, int32
  offsets. Unlike the dma_gather ucode this needs no Q7 library reload
  (~10us), so descriptor generation starts as soon as the offsets land.
  Each 128-row chunk is transposed on the PE (identity matmul) before
  feeding the weight-stationary pm matmuls, and each chunk's 32KB output
  slice is copied out of PSUM and stored independently so the last
  chunk's tail is minimal.

  Everything else is restructured around weight-stationary matmuls in
  bf16 (the harness gate is 2e-2; bf16 keeps rel err ~4e-3):
    - pm^T[dout, prow] accumulates stage/wafer/dyn contributions with the
      (tiny) weights as the stationary lhsT and wide moving rhs. The host
      un-transposes the bf16 result.
    - the robot-arm a_loc path is algebraically folded through
      W_concat @ W_robot[0:D] on the host; together with host-staged arm
      rows (64 rows/core) the whole arm embedding is gather-independent
      and completes while the gathers are still in flight.
    - stage rows (encoded_col is only [16,32,128] per core) and all
      scalar preprocessing (remain_prs, dyn vectors) are prepared
      host-side and shipped as small dense inputs.

All per-core variation is data staged through DRAM inputs; the Bass
program is identical on every core.
"""

import numpy as np
import ml_dtypes

import concourse.bass as bass
import concourse.bacc as bacc
import concourse.mybir as mybir
import concourse.tile as tile
from concourse.bass_utils import run_bass_kernel_spmd

B, N, S, P, D = 128, 4096, 32, 64, 128
NORM = 300.0
NCORES = 8
BL = B // NCORES          # local batches per core = 16
NCHUNK = 8                # indirect gathers of 128 rows each
NARM = 2 * BL             # arm rows per core = 32
XCOLS = 8 * 128 + 4 * NARM  # 1024 pm stage + aloc-stage + ns + aloc-wafer + recipe

F32 = mybir.dt.float32
BF16 = mybir.dt.bfloat16
I32 = mybir.dt.int32
BF = ml_dtypes.bfloat16

_prog_cache = None


def _build_program():
    nc = bacc.Bacc("TRN2", target_bir_lowering=False, num_swdge_queues=1,
                   debug=False)

    rows_h = nc.declare_dram_parameter("rows", [BL * N, D], BF16,
                                       isOutput=False)
    xstage_h = nc.declare_dram_parameter("xstage", [128, XCOLS], BF16,
                                         isOutput=False)
    wstack_h = nc.declare_dram_parameter("wstack", [128, 7, D], BF16,
                                         isOutput=False)
    vecs_h = nc.declare_dram_parameter("vecs", [1, 1472], BF16, isOutput=False)
    idx_h = nc.declare_dram_parameter("idx", [128, NCHUNK], I32, isOutput=False)

    out_pm_h = nc.declare_dram_parameter("out_pm", [128, 8 * 128], BF16,
                                         isOutput=True)
    out_arm_h = nc.declare_dram_parameter("out_arm", [NARM, D], BF16,
                                          isOutput=True)

    with tile.TileContext(nc) as tc:
        with (
            tc.tile_pool(name="consts", bufs=1) as cpool,
            tc.tile_pool(name="gathers", bufs=1) as gpool,
            tc.tile_pool(name="outs", bufs=1) as opool,
            tc.tile_pool(name="ps_pm", bufs=1, space="PSUM") as ps_pm,
            tc.tile_pool(name="ps_tp", bufs=1, space="PSUM") as ps_tp,
            tc.tile_pool(name="ps_arm", bufs=1, space="PSUM") as ps_arm,
        ):
            # ---- idx first, then the 8 indirect row-gathers (mainline SWDGE,
            # no gpsimd library, no reload) ----
            idx = cpool.tile([128, NCHUNK], I32, name="idx")
            nc.sync.dma_start(out=idx[:], in_=idx_h[:])

            gx = []
            for c in range(NCHUNK):
                g = gpool.tile([128, D], BF16, name=f"g{c}", uniquify=False)
                nc.gpsimd.indirect_dma_start(
                    out=g[:], out_offset=None,
                    in_=rows_h[:],
                    in_offset=bass.IndirectOffsetOnAxis(
                        ap=idx[:, c : c + 1], axis=0),
                )
                gx.append(g[:])

            # ---- dense loads ALL behind idx on the SP queue: idx's 128 tiny
            # descriptors then own the DMA engines exclusively, making the
            # gather-gen start time deterministic-minimal (the loads have
            # ~9us of slack before the PE needs them) ----
            wsb = cpool.tile([128, 7, D], BF16, name="wsb")
            nc.sync.dma_start(out=wsb[:], in_=wstack_h[:])
            vecs = cpool.tile([1, 1472], BF16, name="vecs")
            nc.sync.dma_start(out=vecs[:], in_=vecs_h[:])
            xst = cpool.tile([128, XCOLS], BF16, name="xst")
            nc.sync.dma_start(out=xst[:], in_=xstage_h[:])

            w_cs = wsb[:, 0, :]       # W_concat stage segment   [d, dout]
            w_cw = wsb[:, 1, :]       # W_concat wafer segment
            w_rw = wsb[:, 2, :]       # W_robot wafer segment
            w_rn = wsb[:, 3, :]       # W_robot next-stage segment
            w_fs = wsb[:, 4, :]       # W_concat[0:D]   @ W_robot[0:D]
            w_fw = wsb[:, 5, :]       # W_concat[D:2D]  @ W_robot[0:D]
            ident = wsb[:, 6, :]      # identity for PE transposes
            rflat = vecs[:, 0:1024]           # remain_prs, pmT col order
            rfa = vecs[:, 1024:1056]          # remain_prs at each arm's loc PM
            flag = vecs[:, 1056:1088]         # 1.0 where arm loc == P+1
            v_dyn = vecs[:, 1088:1216]        # W_dyn[0] @ W_concat[2D:3D]
            v_dyn_rl = vecs[:, 1216:1344]     # v_dyn @ W_robot[0:D]
            wrl_sum = vecs[:, 1344:1472]      # column sums of W_robot[0:D]

            # ---- PE program: all gather-independent matmuls first ----
            pm_sb = opool.tile([128, 8 * 128], BF16, name="pm_sb")
            pmp = [ps_pm.tile([128, 512], F32, name=f"pmp{h}", tag=f"pmp{h}")
                   for h in range(2)]
            armp = ps_arm.tile([NARM, D], F32, name="armp", tag="armp")

            for h in range(2):
                cols = slice(h * 512, (h + 1) * 512)
                nc.tensor.matmul(pmp[h][:], lhsT=w_cs, rhs=xst[:, cols],
                                 start=True, stop=False)
                nc.tensor.matmul(pmp[h][:], lhsT=v_dyn, rhs=rflat[:, cols],
                                 start=False, stop=False)
            nc.tensor.matmul(armp[:], lhsT=xst[:, 1024:1056], rhs=w_fs,
                             start=True, stop=False)
            nc.tensor.matmul(armp[:], lhsT=xst[:, 1056:1088], rhs=w_rn,
                             start=False, stop=False)
            nc.tensor.matmul(armp[:], lhsT=rfa, rhs=v_dyn_rl,
                             start=False, stop=False)
            nc.tensor.matmul(armp[:], lhsT=xst[:, 1088:1120], rhs=w_fw,
                             start=False, stop=False)
            nc.tensor.matmul(armp[:], lhsT=xst[:, 1120:1152], rhs=w_rw,
                             start=False, stop=False)
            nc.tensor.matmul(armp[:], lhsT=flag, rhs=wrl_sum,
                             start=False, stop=True)
            arm_sb = opool.tile([NARM, D], BF16, name="arm_sb")
            nc.vector.tensor_copy(out=arm_sb[:], in_=armp[:])
            nc.scalar.dma_start(out=out_arm_h[:], in_=arm_sb[:])

            # ---- gather-dependent: transpose each chunk on the PE, then
            # accumulate its wafer contribution; per-chunk psum copy + 32KB
            # store so the last chunk's tail is as short as possible (xt
            # copies on ACT, pm copies on DVE, stores alternate queues) ----
            for c in range(NCHUNK):
                h, qq = divmod(c, 4)
                tp = ps_tp.tile([128, D], BF16, name=f"tp{c}", tag=f"tp{c % 2}")
                nc.tensor.transpose(out=tp[:], in_=gx[c], identity=ident)
                xt = gpool.tile([128, D], BF16, name=f"xt{c}", uniquify=False)
                nc.scalar.copy(out=xt[:], in_=tp[:])
                nc.tensor.matmul(
                    pmp[h][:, qq * 128 : (qq + 1) * 128], lhsT=w_cw,
                    rhs=xt[:], start=False, stop=(qq == 3),
                    skip_group_check=True,
                )
                cols = slice(c * 128, (c + 1) * 128)
                nc.vector.tensor_copy(out=pm_sb[:, cols],
                                      in_=pmp[h][:, qq * 128 : (qq + 1) * 128])
                if c % 2 == 0:
                    nc.sync.dma_start(out=out_pm_h[:, cols], in_=pm_sb[:, cols])
                else:
                    nc.scalar.dma_start(out=out_pm_h[:, cols], in_=pm_sb[:, cols])

    nc.compile()
    return nc


def _get_program():
    global _prog_cache
    if _prog_cache is None:
        _prog_cache = _build_program()
    return _prog_cache


def _prep_core(c, rows_bf, col_bf, remain, W, loc_hold_wafer, loc_stage,
               robot_arm1_loc, robot_arm2_loc, arm1_recipe, arm2_recipe,
               arm1_next_stage, arm2_next_stage):
    b0 = c * BL
    bs = slice(b0, b0 + BL)

    rows = rows_bf[bs].reshape(BL * N, D)

    lhw = np.where(loc_hold_wafer[bs] >= 0, loc_hold_wafer[bs], 0).astype(np.int64)
    lst = loc_stage[bs].astype(np.int64)                      # in [1, S]
    rem = remain[bs]                                          # [BL, P] f32
    loc = np.stack([robot_arm1_loc[bs, 0], robot_arm2_loc[bs, 0]], 1).astype(np.int64)
    rec = np.stack([arm1_recipe[bs, 0], arm2_recipe[bs, 0]], 1).astype(np.int64)
    nst = np.stack([arm1_next_stage[bs, 0], arm2_next_stage[bs, 0]], 1).astype(np.int64)

    locv = (loc >= 1) & (loc <= P)                            # [BL, 2] valid pm loc
    locp = np.where(locv, loc - 1, 0)                         # the arm's PM index
    recv = rec >= 0
    lbi = np.arange(BL)[:, None]

    # gather idx [128, 8] int32: chunk c partition p = pmT col c*128+p
    lb_of = np.arange(BL).repeat(P)                           # col -> lb
    p_of = np.tile(np.arange(P), BL)                          # col -> pm
    idxfull = lb_of * N + lhw[lb_of, p_of]                    # [1024]
    idx = np.ascontiguousarray(
        idxfull.reshape(NCHUNK, 128).T.astype(np.int32))      # [128, 8]

    # xstageT [128, 1152]
    colc = col_bf[bs]
    rowc = rows_bf[bs]
    xst = np.zeros((XCOLS, D), BF)
    xst[0:1024] = colc[lbi, lst - 1].reshape(1024, D)
    xst[1024:1056] = np.where(locv[:, :, None],
                              colc[lbi, lst[lbi, locp] - 1], 0).reshape(NARM, D)
    nsv = (nst >= 1) & (nst <= S)
    xst[1056:1088] = np.where(nsv[:, :, None],
                              colc[lbi, np.where(nsv, nst - 1, 0)], 0
                              ).reshape(NARM, D)
    xst[1088:1120] = np.where(locv[:, :, None],
                              rowc[lbi, lhw[lbi, locp]], 0).reshape(NARM, D)
    xst[1120:1152] = np.where(recv[:, :, None],
                              rowc[lbi, np.where(recv, rec, 0)], 0
                              ).reshape(NARM, D)

    vecs = np.zeros((1, 1472), BF)
    vecs[0, 0:1024] = rem.reshape(-1).astype(BF)
    vecs[0, 1024:1056] = np.where(locv, rem[lbi, locp], 0).reshape(-1).astype(BF)
    vecs[0, 1056:1088] = (loc == P + 1).reshape(-1).astype(BF)
    vecs[0, 1088:1472] = W["vec3"]

    return {
        "rows": rows,
        "xstage": np.ascontiguousarray(xst.T),
        "wstack": W["wstack"],
        "vecs": vecs,
        "idx": idx,
    }


def make_in_maps(inputs):
    inputs = {k: np.asarray(v) for k, v in inputs.items()}
    Wc = inputs["W_concat"].astype(np.float32)
    Wr = inputs["W_robot"].astype(np.float32)
    Wd = inputs["W_dyn"].astype(np.float32)
    w_rl = Wr[0:D]

    wstack = np.ascontiguousarray(
        np.stack(
            [Wc[0:D], Wc[D : 2 * D], Wr[D : 2 * D], Wr[2 * D : 3 * D],
             Wc[0:D] @ w_rl, Wc[D : 2 * D] @ w_rl, np.eye(D, dtype=np.float32)],
            axis=1,
        )
    ).astype(BF)                                              # [128, 7, D]
    v_dyn = (Wd[0:1] @ Wc[2 * D : 3 * D]).reshape(D)
    vec3 = np.concatenate([v_dyn, v_dyn @ w_rl, w_rl.sum(0)]).astype(BF)
    W = {"wstack": wstack, "vec3": vec3}

    rows_bf = inputs["encoded_row"].astype(BF)                # [B, N, D]
    col_bf = inputs["encoded_col"].astype(BF)                 # [B, S, D]
    clk = inputs["clock"].astype(np.float32)                  # [B, 1]
    lpet = inputs["loc_process_end_time"].astype(np.float32)  # [B, P]
    remain = np.maximum(lpet - clk, 0.0) / NORM               # [B, P]

    ks = ("loc_hold_wafer", "loc_stage", "robot_arm1_loc", "robot_arm2_loc",
          "arm1_recipe", "arm2_recipe", "arm1_next_stage", "arm2_next_stage")
    return [
        _prep_core(c, rows_bf, col_bf, remain, W, **{k: inputs[k] for k in ks})
        for c in range(NCORES)
    ]


def assemble_output(res):
    out = np.empty((B, P + 2, D), np.float32)
    for c in range(NCORES):
        pmT = res[c]["out_pm"].astype(np.float32)             # [128, 1024]
        pm = pmT.reshape(D, 8, 2, P).transpose(1, 2, 3, 0).reshape(BL, P, D)
        out[c * BL : (c + 1) * BL, 0:P, :] = pm
        out[c * BL : (c + 1) * BL, P:, :] = (
            res[c]["out_arm"].astype(np.float32).reshape(BL, 2, D)
        )
    return out


def kernel(**inputs):
    in_maps = make_in_maps(inputs)
    nc = _get_program()
    res = run_bass_kernel_spmd(nc, in_maps, list(range(NCORES))).results
    return assemble_output(res)

